# revision 12
# baseline (speedup 1.0000x reference)
"""Trainium2 Bass kernel for nn_AnchorPlusLoss (8 NeuronCores, data-parallel).

Math (per batch b):
  x = embedding; x[..., :2] += abs_coords          # coords fold into first 2 dims
  ssq[i,j] = ||x_i - x_j||^2 + EPS  (Gram matmul, bf16 hi/lo split)
  sim = sigmoid(5 - sqrt(ssq))  approximated WITHOUT sqrt/exp:
      logit(ssq) = 5 - sqrt(ssq) ~ C*arctan(A*ssq + B) + E   (4-param fit,
      max abs sigma-domain err 2.5e-3 over the off-diagonal range d>=4)
  em1 = expm1(sim) ~ AL1*sim                                  (deg-1 fit; the
      masked/unmasked halves share elements so fit errors cancel in negsum)
  negsum_i = nneg_i + AL1*(sum_j sim - sum_m sim - diag corr)
  loss_i = npos_i*log(negsum_i) - sum_m sim + (npos_i + AL1*sum_m sim)/negsum_i
  (end-to-end rel err vs reference ~4e-4, tolerance 2e-2)

Device layout (per core = one (batch, 512-row half)): partitions = i
(4 tiles of 128 rows), free axis = j (1024). Engine split per iteration:
  PE:      8 matmuls  -> psum ssq [128, 4096] f32          (~3.4 us @mid)
  ScalarE: arctan x2 (2048-wide, from PSUM) -> atn f32
           sigmoid x2 -> st bf16; both funcs share ONE activation table set
           (sigmoid_and_others) => no per-iteration table loads  (~7.6 us)
  DVE:     Sm per tile via fused scalar_tensor_tensor+accum((st*1)*mask);
           Sall via a bf16 halving tree + one tensor_reduce(X)   (~7 us)
  out:     acc [128, 8] f32 (Sall/Sm per tile) DMA'd once at the end.
Sums over j are per-partition free-axis accumulator outputs -- the old
per-element Sqrt/Tanh/Exp passes and 24 reduction matmuls are gone.
"""

import sys
import time

sys.path.insert(0, "/opt/trn_rl_repo")

import numpy as np
import ml_dtypes

N_CORES = 8
B, N, E = 4, 1024, 32
ROWS = 512          # rows (i) per core
P = 128             # partitions
TI = ROWS // P      # 4 i-tiles per core
K = 34              # contraction: 32 emb + 2 augmentation rows
K2 = 3 * K          # hi/lo packed contraction
EPS = 0.25          # added to ssq so the diagonal never goes negative
SMV_W = ROWS + N    # packed operand width: mv2 (512) | s2 (1024)

# logit(u) = 5 - sqrt(u) ~ C_*arctan(A_*u + B_) + E_  (minimax fit, u >= 16)
A_ = 0.047240052
B_ = 4.49660097
C_ = -70.9121647
E_ = 99.0348340
AL1 = 1.3196475487382737   # expm1(s) ~ AL1*s over s in [0, 0.70]

NV = 9    # DVE instructions per iteration (vsem increments)
NBS = 4   # smv input buffers (prefetch depth)
NBM = 4   # mask input buffers
NBT = 3   # sigmoid-output buffers

_nc_cache = {}
_runner_cache = {}


def _build_body(nc, mybir, ext, sb, sems, iters, variant="full"):
    """Emit all iterations inside ONE Block: per-iteration Blocks would put
    an all-engine barrier between iterations, serializing the pipeline.
    Semaphores are shared across iterations with monotonic per-iteration
    threshold offsets. Cross-iteration WAR hazards are covered by
    double-buffered inputs (smv/mask/st) plus tsem/vsem waits on the
    producer engines; same-engine hazards rely on in-order execution.

    variant (for bottleneck experiments): "full"; "nodve" drops the DVE
    stage; "noact" drops the activations; "nomask"/"nosmv" load that input
    only once; "mm" = DMAs+matmuls only; "dma" = DMAs only."""
    AF = mybir.ActivationFunctionType
    A = mybir.AluOpType
    smv_ext, m_ext, acc_ext = ext
    (smv_sb, m_sb, atn_sb, st_sb, ms_sb, scr_sb, acc_sb, ps) = sb
    (dssem, dmsem, tsem, asem, ssem, vsem, osem, psem, constsem) = sems
    HW = 2048  # half width (2 i-tiles) for the big activations

    do_mm = variant in ("full", "nodve", "noact", "nomask", "nosmv", "mm",
                       "nodma", "f16", "pool", "stt")
    do_act = variant in ("full", "nodve", "nomask", "nosmv", "nodma", "f16",
                        "pool", "stt")
    do_dve = variant in ("full", "noact", "nomask", "nosmv", "nodma", "f16",
                        "pool", "stt")
    mask_every = variant not in ("nomask", "nodma")
    smv_every = variant not in ("nosmv", "nodma")
    use_pool = variant == "pool"
    use_stt = variant == "stt"

    def dve_ready(it):
        # threshold proving iteration `it`'s DVE block is complete
        return 12 * (it + 1)

    with nc.Block() as block:

        @block.sync
        def _(sync):
            # iteration 0's inputs, then per-iteration prefetch of it+1
            sync.dma_start(smv_sb[0][:, :], smv_ext[:, :]).then_inc(dssem, 16)
            sync.dma_start(m_sb[0][:, :], m_ext[:, :]).then_inc(dmsem, 16)
            for it in range(iters - 1):
                if smv_every:
                    # buffer (it+1)%NBS last read by PE(it+1-NBS)
                    thr = 8 * (it + 2 - NBS)
                    if thr > 0:
                        if do_mm:
                            sync.wait_ge(tsem, thr)
                        else:
                            sync.wait_ge(dssem, 2 * thr)
                    sync.dma_start(
                        smv_sb[(it + 1) % NBS][:, :], smv_ext[:, :]
                    ).then_inc(dssem, 16)
                if mask_every:
                    # buffer (it+1)%NBM last read by DVE(it+1-NBM) instr 1
                    k = it + 1 - NBM
                    if k >= 0:
                        if do_dve:
                            sync.wait_ge(vsem, NV * k + 1)
                        else:
                            sync.wait_ge(dmsem, 16 * (k + 1))
                    sync.dma_start(
                        m_sb[(it + 1) % NBM][:, :], m_ext[:, :]
                    ).then_inc(dmsem, 16)
            # epilogue: single output DMA once the last stage is done
            if do_dve:
                sync.wait_ge(vsem, NV * iters)
            elif do_act:
                sync.wait_ge(ssem, 2 * iters)
            elif do_mm:
                sync.wait_ge(tsem, 8 * iters)
            else:
                sync.wait_ge(dssem, 16 * (iters - 1) if smv_every else 16)
                sync.wait_ge(dmsem, 16 * (iters - 1) if mask_every else 16)
            sync.dma_start(acc_ext[:, :], acc_sb[:, :]).then_inc(osem, 16)
            sync.wait_ge(osem, 16)

        if do_mm:

            @block.tensor
            def _(tensor):
                for it in range(iters):
                    buf = it % NBS if smv_every else 0
                    tensor.wait_ge(
                        dssem, 16 * (it + 1) if smv_every else 16
                    )
                    mv2 = smv_sb[buf][:, 0:ROWS]
                    s2 = smv_sb[buf][:, ROWS : ROWS + N]
                    for h in range(2):
                        # WAR: ps half h read by iteration it-1's arctan h
                        thr = 2 * (it - 1) + h + 1
                        if do_act and thr > 0:
                            tensor.wait_ge(asem, thr)
                        for t in (2 * h, 2 * h + 1):
                            for jh in range(2):
                                off = 1024 * t + 512 * jh
                                tensor.matmul(
                                    ps[:, off : off + 512],
                                    mv2[:, P * t : P * (t + 1)],
                                    s2[:, 512 * jh : 512 * (jh + 1)],
                                    start=True,
                                    stop=True,
                                ).then_inc(tsem)

        if do_act:

            @block.scalar
            def _(scalar):
                scalar.wait_ge(constsem, 2)
                for it in range(iters):
                    buf = it % NBT
                    for h in range(2):
                        scalar.wait_ge(tsem, 8 * it + 4 * (h + 1))
                        scalar.activation(
                            atn_sb[:, h * HW : (h + 1) * HW],
                            ps[:, h * HW : (h + 1) * HW],
                            AF.Arctan,
                            bias=B_,
                            scale=A_,
                        ).then_inc(asem)
                    if do_dve and it >= NBT:
                        # WAR: st[buf] was read by iteration it-NBT's DVE
                        scalar.wait_ge(vsem, NV * (it - NBT + 1))
                    for h in range(2):
                        scalar.activation(
                            st_sb[buf][:, h * HW : (h + 1) * HW],
                            atn_sb[:, h * HW : (h + 1) * HW],
                            AF.Sigmoid,
                            bias=E_,
                            scale=C_,
                        ).then_inc(ssem)

        if use_pool:

            @block.gpsimd
            def _(gp):
                # Pool multiplies mask*st for tiles 2,3 (idle engine offload)
                for it in range(iters):
                    buf = it % NBT
                    mbuf = it % NBM if mask_every else 0
                    gp.wait_ge(dmsem, 16 * (it + 1) if mask_every else 16)
                    gp.wait_ge(ssem, 2 * (it + 1))
                    if it > 0:
                        # WAR: DVE(it-1) ms-tree level1 (6th instr) read ms
                        gp.wait_ge(vsem, NV * (it - 1) + 6)
                    gp.tensor_tensor(
                        out=ms_sb[:, 2048:4096],
                        in0=m_sb[mbuf][:, 2048:4096],
                        in1=st_sb[buf][:, 2048:4096],
                        op=A.mult,
                    ).then_inc(psem)

        if do_dve:

            @block.vector
            def _(vector):
                for it in range(iters):
                    buf = it % NBT
                    mbuf = it % NBM if mask_every else 0
                    vector.wait_ge(dmsem, 16 * (it + 1) if mask_every else 16)
                    if do_act:
                        vector.wait_ge(ssem, 2 * (it + 1))
                    elif do_mm:
                        vector.wait_ge(tsem, 8 * (it + 1))
                    st = st_sb[buf]
                    st3 = st.rearrange("p (t j) -> p t j", t=TI)
                    m3 = m_sb[mbuf]
                    ms3 = ms_sb.rearrange("p (t j) -> p t j", t=TI)
                    a1 = scr_sb[:, 0:2048].rearrange("p (t j) -> p t j", t=TI)
                    a2 = scr_sb[:, 2048:3072].rearrange(
                        "p (t j) -> p t j", t=TI
                    )
                    a3 = scr_sb[:, 3072:3584].rearrange(
                        "p (t j) -> p t j", t=TI
                    )
                    if use_stt:
                        # Sm per tile in ONE fused instr: accum((st*1)*m);
                        # the ms tensor and its tree disappear entirely
                        for t in range(TI):
                            sl = slice(1024 * t, 1024 * (t + 1))
                            vector.scalar_tensor_tensor(
                                out=scr_sb[:, sl],
                                in0=st[:, sl],
                                scalar=1.0,
                                in1=m3[:, sl],
                                op0=A.mult,
                                op1=A.mult,
                                accum_out=acc_sb[:, TI + t : TI + t + 1],
                            ).then_inc(vsem)
                        # Sall via the halving tree (unmasked path)
                        a1s = a1
                        vector.tensor_tensor(
                            out=a1s[:, :, :],
                            in0=st3[:, :, 0:512],
                            in1=st3[:, :, 512:1024],
                            op=A.add,
                        ).then_inc(vsem)
                        vector.tensor_tensor(
                            out=a2[:, :, :],
                            in0=a1s[:, :, 0:256],
                            in1=a1s[:, :, 256:512],
                            op=A.add,
                        ).then_inc(vsem)
                        vector.tensor_tensor(
                            out=a3[:, :, :],
                            in0=a2[:, :, 0:128],
                            in1=a2[:, :, 128:256],
                            op=A.add,
                        ).then_inc(vsem)
                        vector.tensor_reduce(
                            acc_sb[:, 0:TI],
                            a3[:, :, :],
                            mybir.AxisListType.X,
                            A.add,
                        ).then_inc(vsem)
                        # keep vsem at NV increments per iteration
                        vector.memset(scr_sb[0:1, 0:1], 0).then_inc(vsem)
                        continue
                    # ms = mask * st (tiles 0-1 here; 2-3 on Pool if enabled)
                    if use_pool:
                        vector.tensor_tensor(
                            out=ms_sb[:, 0:2048],
                            in0=m3[:, 0:2048],
                            in1=st[:, 0:2048],
                            op=A.mult,
                        ).then_inc(vsem)
                    else:
                        vector.tensor_tensor(
                            out=ms_sb[:, :], in0=m3[:, :], in1=st[:, :],
                            op=A.mult,
                        ).then_inc(vsem)
                    # per-tile free-axis sums via halving tree + one
                    # innermost-axis tensor_reduce for all 4 tiles at once
                    for (src3, col) in ((st3, 0), (ms3, TI)):
                        if use_pool and src3 is ms3:
                            vector.wait_ge(psem, it + 1)
                        vector.tensor_tensor(
                            out=a1[:, :, :],
                            in0=src3[:, :, 0:512],
                            in1=src3[:, :, 512:1024],
                            op=A.add,
                        ).then_inc(vsem)
                        vector.tensor_tensor(
                            out=a2[:, :, :],
                            in0=a1[:, :, 0:256],
                            in1=a1[:, :, 256:512],
                            op=A.add,
                        ).then_inc(vsem)
                        vector.tensor_tensor(
                            out=a3[:, :, :],
                            in0=a2[:, :, 0:128],
                            in1=a2[:, :, 128:256],
                            op=A.add,
                        ).then_inc(vsem)
                        vector.tensor_reduce(
                            acc_sb[:, col : col + TI],
                            a3[:, :, :],
                            mybir.AxisListType.X,
                            A.add,
                        ).then_inc(vsem)


def _build_nc(iters=1, variant="full"):
    import concourse.bass as bass
    import concourse.mybir as mybir

    nc = bass.Bass()

    # Register const APs for the two activation biases (framework only
    # pre-registers 0.0/1.0). Only ScalarE reads these; a semaphore from the
    # memsets suffices (no all-engine barrier).
    constsem = nc.alloc_semaphore("constsem")
    for v in (B_, E_):
        t = nc.alloc_sbuf_tensor(f"const-f32-{v}", [128, 1], mybir.dt.float32)
        nc.gpsimd.memset(t.ap(), v).then_inc(constsem)
        nc.const_aps.aps[(mybir.dt.float32, v)] = t.ap()

    f32, bf16 = mybir.dt.float32, mybir.dt.bfloat16
    atn_dt = mybir.dt.float16 if variant == "f16" else f32
    ext = (
        nc.declare_dram_parameter("smv", [K2, SMV_W], bf16, isOutput=False),
        nc.declare_dram_parameter("mask", [P, TI * N], bf16, isOutput=False),
        nc.declare_dram_parameter("acc", [P, 2 * TI], f32, isOutput=True),
    )
    smv_sb = [
        nc.alloc_sbuf_tensor(f"smv{i}", [K2, SMV_W], bf16) for i in range(NBS)
    ]
    m_sb = [
        nc.alloc_sbuf_tensor(f"m{i}", [P, TI * N], bf16) for i in range(NBM)
    ]
    st_sb = [
        nc.alloc_sbuf_tensor(f"st{i}", [P, TI * N], bf16) for i in range(NBT)
    ]
    sb = (
        [t.ap() for t in smv_sb],
        [t.ap() for t in m_sb],
        nc.alloc_sbuf_tensor("atn", [P, TI * N], atn_dt).ap(),
        [t.ap() for t in st_sb],
        nc.alloc_sbuf_tensor("ms", [P, TI * N], bf16).ap(),
        nc.alloc_sbuf_tensor("scr", [P, TI * N], bf16).ap(),
        nc.alloc_sbuf_tensor("acc_sb", [P, 2 * TI], f32).ap(),
        nc.alloc_psum_tensor("ps", [P, TI * N], f32).ap(),
    )
    # fix list-of-AP indexing helpers
    smv_l, m_l, atn, st_l, ms, scr, acc, ps = sb

    class _Pair:
        def __init__(self, aps):
            self.aps = aps

        def __getitem__(self, i):
            return self.aps[i]

    sb = (_Pair(smv_l), _Pair(m_l), atn, _Pair(st_l), ms, scr, acc, ps)

    sems = tuple(
        nc.alloc_semaphore(n)
        for n in ("dssem", "dmsem", "tsem", "asem", "ssem", "vsem", "osem",
                  "psem")
    ) + (constsem,)
    (dssem, dmsem, tsem, asem, ssem, vsem, osem, psem, constsem) = sems

    _build_body(nc, mybir, ext, sb, sems, iters, variant=variant)

    return nc


def _get_nc(iters=1, variant="full"):
    key = (iters, variant)
    if key not in _nc_cache:
        _nc_cache[key] = _build_nc(iters, variant)
    return _nc_cache[key]


def _split_bf16(a):
    hi = a.astype(ml_dtypes.bfloat16)
    lo = (a - hi.astype(np.float64)).astype(ml_dtypes.bfloat16)
    return hi, lo


def _host_prep(embedding, abs_coords, patch_mask):
    """Build per-core input maps."""
    x = embedding.astype(np.float64).copy()  # [B,N,E]
    x[:, :, :2] += abs_coords.astype(np.float64)
    r = np.einsum("bne,bne->bn", x, x)  # [B,N]

    in_maps = []
    for c in range(N_CORES):
        b, i0 = c // 2, ROWS * (c % 2)
        xt = x[b].T  # [E, N]
        s = np.empty((K, N), np.float64)
        s[:E] = xt
        s[E] = r[b]
        s[E + 1] = 1.0
        mv = np.empty((K, ROWS), np.float64)
        mv[:E] = -2.0 * xt[:, i0 : i0 + ROWS]
        mv[E] = 1.0
        mv[E + 1] = r[b, i0 : i0 + ROWS] + EPS
        s_hi, s_lo = _split_bf16(s)
        mv_hi, mv_lo = _split_bf16(mv)
        s2 = np.concatenate([s_hi, s_hi, s_lo], axis=0)      # [K2, N]
        mv2 = np.concatenate([mv_hi, mv_lo, mv_hi], axis=0)  # [K2, ROWS]
        smv = np.concatenate([mv2, s2], axis=1)              # [K2, SMV_W]
        # m_sb[p, 1024*t + j] = mask[b, i0 + 128*t + p, j]
        m = (
            patch_mask[b][i0 : i0 + ROWS, :]
            .reshape(TI, P, N)
            .transpose(1, 0, 2)
            .reshape(P, TI * N)
            .astype(ml_dtypes.bfloat16)
        )
        in_maps.append(
            {"smv": np.ascontiguousarray(smv), "mask": np.ascontiguousarray(m)}
        )
    return in_maps


def _host_combine(results, patch_mask):
    """Per-row logs + final sum on host (4096 rows, trivial)."""
    # device diagonal value: ssq_ii = EPS exactly
    s_ii = 1.0 / (1.0 + np.exp(-(C_ * np.arctan(A_ * EPS + B_) + E_)))

    total = 0.0
    for c in range(N_CORES):
        b, i0 = c // 2, ROWS * (c % 2)
        acc = results[c]["acc"].astype(np.float64)  # [128, 8]
        mrows = patch_mask[b][i0 : i0 + ROWS, :].astype(np.float64)
        npos_all = mrows.sum(axis=1)                     # [512] in row order
        diag_all = np.diagonal(patch_mask[b])[i0 : i0 + ROWS].astype(
            np.float64
        )
        for t in range(TI):
            sall = acc[:, t]
            sm = acc[:, TI + t]
            rows = slice(128 * t, 128 * (t + 1))
            npos = npos_all[rows]
            diag = diag_all[rows]
            sneg = sall - sm - (1.0 - diag) * s_ii
            negsum = (N - npos - (1.0 - diag)) + AL1 * sneg
            L = np.log(negsum)
            total += (npos * L - sm + (npos + AL1 * sm) / negsum).sum()
    return total


def _make_runner(nc, in_maps):
    """Persistent jitted SPMD runner mirroring bass2jax.run_bass_via_pjrt.

    Returns f() -> list[dict[name, np.ndarray]]; repeated calls reuse the
    compiled executable so wall-clock deltas reflect device execution.
    """
    import jax
    from jax.sharding import Mesh, PartitionSpec, NamedSharding
    from jax.experimental.shard_map import shard_map
    import concourse.mybir as mybir
    from concourse import bass2jax

    bass2jax.install_neuronx_cc_hook()
    nc.finalize()

    partition_name = nc.partition_id_tensor.name if nc.partition_id_tensor else None
    in_names, out_names, out_avals, zero_outs = [], [], [], []
    for alloc in nc.m.functions[0].allocations:
        if not isinstance(alloc, mybir.MemoryLocationSet):
            continue
        name = alloc.memorylocations[0].name
        if alloc.kind == "ExternalInput":
            if name != partition_name:
                in_names.append(name)
        elif alloc.kind == "ExternalOutput":
            shape = tuple(alloc.tensor_shape)
            dtype = mybir.dt.np(alloc.dtype)
            out_names.append(name)
            out_avals.append(jax.core.ShapedArray(shape, dtype))
            zero_outs.append(np.zeros(shape, dtype))
    n_params = len(in_names)
    n_outs = len(out_avals)
    in_names_all = in_names + out_names
    if partition_name is not None:
        in_names_all.append(partition_name)

    def _body(*args):
        operands = list(args)
        if partition_name is not None:
            operands.append(bass2jax.partition_id_tensor())
        outs = bass2jax._bass_exec_p.bind(
            *operands,
            out_avals=tuple(out_avals),
            in_names=tuple(in_names_all),
            out_names=tuple(out_names),
            lowering_input_output_aliases=(),
            sim_require_finite=True,
            sim_require_nnan=True,
            nc=nc,
        )
        return tuple(outs)

    devices = jax.devices()[:N_CORES]
    mesh = Mesh(np.asarray(devices), ("core",))
    in_specs = (PartitionSpec("core"),) * (n_params + n_outs)
    out_specs = (PartitionSpec("core"),) * len(out_names)
    sharded = jax.jit(
        shard_map(
            _body, mesh=mesh, in_specs=in_specs, out_specs=out_specs, check_rep=False
        ),
        keep_unused=True,
    )
    per_core = [[np.asarray(m[name]) for name in in_names] for m in in_maps]
    concat_in = [
        np.concatenate([per_core[c][i] for c in range(N_CORES)], axis=0)
        for i in range(n_params)
    ]
    shard = NamedSharding(mesh, PartitionSpec("core"))
    concat_in_dev = [jax.device_put(a, shard) for a in concat_in]

    concat_zeros_dev = [
        jax.device_put(
            np.zeros((N_CORES * z.shape[0], *z.shape[1:]), z.dtype), shard
        )
        for z in zero_outs
    ]

    def run(fetch=True, block=True):
        out_arrs = sharded(*concat_in_dev, *concat_zeros_dev)
        if not fetch:
            if block:
                jax.block_until_ready(out_arrs)
                return None
            return out_arrs
        out_arrs = [np.asarray(a) for a in out_arrs]
        return [
            {
                name: out_arrs[i].reshape(N_CORES, *out_avals[i].shape)[c]
                for i, name in enumerate(out_names)
            }
            for c in range(N_CORES)
        ]

    return run


def _run(embedding, abs_coords, patch_mask, trace=False, variant="stt"):
    from concourse.bass_utils import run_bass_kernel_spmd

    nc = _get_nc(1, variant)
    in_maps = _host_prep(embedding, abs_coords, patch_mask)
    res = run_bass_kernel_spmd(
        nc, in_maps, core_ids=list(range(N_CORES)), trace=trace
    )
    total = _host_combine(res.results, patch_mask)
    return np.asarray(total, dtype=np.float32), res


def bench(embedding, abs_coords, patch_mask, iters=257, variant="stt"):
    """Measure per-iteration HW time: async-queue k executions of an
    iters-looped NEFF, block once; slope over k cancels dispatch noise."""
    import jax

    in_maps = _host_prep(embedding, abs_coords, patch_mask)
    key = (iters, variant)
    if key not in _runner_cache:
        _runner_cache[key] = _make_runner(_get_nc(iters, variant), in_maps)
    f = _runner_cache[key]
    out = f()  # warm-up + correctness output

    def batch(k):
        outs = None
        t0 = time.perf_counter()
        for _ in range(k):
            outs = f(fetch=False, block=False)
        jax.block_until_ready(outs)
        return time.perf_counter() - t0

    batch(2)
    t2 = min(batch(2) for _ in range(4))
    t10 = min(batch(10) for _ in range(4))
    ns = (t10 - t2) / (8 * iters) * 1e9
    return ns, out


def kernel(embedding, abs_coords, patch_mask):
    emb = np.asarray(embedding)
    coords = np.asarray(abs_coords)
    mask = np.asarray(patch_mask)
    # retry guard: first executions on this fleet occasionally glitch
    # transiently -- either a non-finite result or a device-unrecoverable
    # exception; both clear on retry
    last_err = None
    out = None
    for attempt in range(4):
        try:
            out, _ = _run(emb, coords, mask)
        except Exception as e:  # device-side transient; back off and retry
            last_err = e
            time.sleep(2.0 * (attempt + 1))
            continue
        if np.isfinite(out):
            return out
    if last_err is not None and out is None:
        raise last_err
    return out


# revision 13
# speedup vs baseline: 1.2423x; 1.2423x over previous
"""Trainium2 Bass kernel for nn_AnchorPlusLoss (8 NeuronCores, data-parallel).

Math (per batch b):
  x = embedding; x[..., :2] += abs_coords          # coords fold into first 2 dims
  ssq[i,j] = ||x_i - x_j||^2 + EPS  (Gram matmul, bf16 hi/lo split)
  sim = sigmoid(5 - sqrt(ssq))  approximated WITHOUT sqrt/exp:
      logit(ssq) = 5 - sqrt(ssq) ~ C*arctan(A*ssq + B) + E   (4-param fit,
      max abs sigma-domain err 2.5e-3 over the off-diagonal range d>=4)
  em1 = expm1(sim) ~ AL1*sim                                  (deg-1 fit; the
      masked/unmasked halves share elements so fit errors cancel in negsum)
  negsum_i = nneg_i + AL1*(sum_j sim - sum_m sim - diag corr)
  loss_i = npos_i*log(negsum_i) - sum_m sim + (npos_i + AL1*sum_m sim)/negsum_i
  (end-to-end rel err vs reference ~4e-4, tolerance 2e-2)

Device layout (per core = one (batch, 512-row half)): partitions = i
(4 tiles of 128 rows), free axis = j (1024). Engine split per iteration:
  PE:      8 matmuls  -> psum ssq [128, 4096] f32          (~3.4 us @mid)
  ScalarE: arctan x2 (2048-wide, from PSUM) -> atn f32
           sigmoid x2 -> st bf16; both funcs share ONE activation table set
           (sigmoid_and_others) => no per-iteration table loads  (~7.6 us)
  DVE:     Sm per tile via fused scalar_tensor_tensor+accum((st*1)*mask);
           Sall via a bf16 halving tree + one tensor_reduce(X)   (~7 us)
  out:     acc [128, 8] f32 (Sall/Sm per tile) DMA'd once at the end.
Sums over j are per-partition free-axis accumulator outputs -- the old
per-element Sqrt/Tanh/Exp passes and 24 reduction matmuls are gone.
"""

import sys
import time

sys.path.insert(0, "/opt/trn_rl_repo")

import numpy as np
import ml_dtypes

N_CORES = 8
B, N, E = 4, 1024, 32
ROWS = 512          # rows (i) per core
P = 128             # partitions
TI = ROWS // P      # 4 i-tiles per core
K = 34              # contraction: 32 emb + 2 augmentation rows
K2 = 3 * K          # hi/lo packed contraction
EPS = 0.25          # added to ssq so the diagonal never goes negative
SMV_W = ROWS + N    # packed operand width: mv2 (512) | s2 (1024)

# logit(u) = 5 - sqrt(u) ~ C_*arctan(A_*u + B_) + E_  (minimax fit, u >= 16)
A_ = 0.047240052
B_ = 4.49660097
C_ = -70.9121647
E_ = 99.0348340
AL1 = 1.3196475487382737   # expm1(s) ~ AL1*s over s in [0, 0.70]

NV = 9    # DVE instructions per iteration (vsem increments)
NBS = 4   # smv input buffers (prefetch depth)
NBM = 4   # mask input buffers
NBT = 4   # sigmoid-output buffers

_nc_cache = {}
_runner_cache = {}


def _build_body(nc, mybir, ext, sb, sems, iters, variant="full"):
    """Emit all iterations inside ONE Block: per-iteration Blocks would put
    an all-engine barrier between iterations, serializing the pipeline.
    Semaphores are shared across iterations with monotonic per-iteration
    threshold offsets. Cross-iteration WAR hazards are covered by
    double-buffered inputs (smv/mask/st) plus tsem/vsem waits on the
    producer engines; same-engine hazards rely on in-order execution.

    variant (for bottleneck experiments): "full"; "nodve" drops the DVE
    stage; "noact" drops the activations; "nomask"/"nosmv" load that input
    only once; "mm" = DMAs+matmuls only; "dma" = DMAs only."""
    AF = mybir.ActivationFunctionType
    A = mybir.AluOpType
    smv_ext, m_ext, acc_ext = ext
    (smv_sb, m_sb, atn_sb, st_sb, ms_sb, scr_sb, acc_sb, ps) = sb
    (dssem, dmsem, tsem, asem, ssem, vsem, osem, psem, constsem) = sems
    HW = 2048  # half width (2 i-tiles) for the big activations

    do_mm = variant in ("full", "nodve", "noact", "nomask", "nosmv", "mm",
                       "nodma", "f16", "pool", "stt")
    do_act = variant in ("full", "nodve", "nomask", "nosmv", "nodma", "f16",
                        "pool", "stt")
    do_dve = variant in ("full", "noact", "nomask", "nosmv", "nodma", "f16",
                        "pool", "stt")
    mask_every = variant not in ("nomask", "nodma")
    smv_every = variant not in ("nosmv", "nodma")
    use_pool = variant == "pool"
    use_stt = variant == "stt"

    def dve_ready(it):
        # threshold proving iteration `it`'s DVE block is complete
        return 12 * (it + 1)

    with nc.Block() as block:

        @block.sync
        def _(sync):
            # iteration 0's inputs, then per-iteration prefetch of it+1
            sync.dma_start(smv_sb[0][:, :], smv_ext[:, :]).then_inc(dssem, 16)
            sync.dma_start(m_sb[0][:, :], m_ext[:, :]).then_inc(dmsem, 16)
            for it in range(iters - 1):
                if smv_every:
                    # buffer (it+1)%NBS last read by PE(it+1-NBS)
                    thr = 8 * (it + 2 - NBS)
                    if thr > 0:
                        if do_mm:
                            sync.wait_ge(tsem, thr)
                        else:
                            sync.wait_ge(dssem, 2 * thr)
                    sync.dma_start(
                        smv_sb[(it + 1) % NBS][:, :], smv_ext[:, :]
                    ).then_inc(dssem, 16)
                if mask_every:
                    # buffer (it+1)%NBM last read by DVE(it+1-NBM) instr 1
                    k = it + 1 - NBM
                    if k >= 0:
                        if do_dve:
                            sync.wait_ge(vsem, NV * k + 1)
                        else:
                            sync.wait_ge(dmsem, 16 * (k + 1))
                    sync.dma_start(
                        m_sb[(it + 1) % NBM][:, :], m_ext[:, :]
                    ).then_inc(dmsem, 16)
            # epilogue: single output DMA once the last stage is done
            if do_dve:
                sync.wait_ge(vsem, NV * iters)
            elif do_act:
                sync.wait_ge(ssem, 2 * iters)
            elif do_mm:
                sync.wait_ge(tsem, 8 * iters)
            else:
                sync.wait_ge(dssem, 16 * (iters - 1) if smv_every else 16)
                sync.wait_ge(dmsem, 16 * (iters - 1) if mask_every else 16)
            sync.dma_start(acc_ext[:, :], acc_sb[:, :]).then_inc(osem, 16)
            sync.wait_ge(osem, 16)

        if do_mm:

            @block.tensor
            def _(tensor):
                for it in range(iters):
                    buf = it % NBS if smv_every else 0
                    tensor.wait_ge(
                        dssem, 16 * (it + 1) if smv_every else 16
                    )
                    mv2 = smv_sb[buf][:, 0:ROWS]
                    s2 = smv_sb[buf][:, ROWS : ROWS + N]
                    for h in range(2):
                        # WAR: ps half h read by iteration it-1's arctan h
                        thr = 2 * (it - 1) + h + 1
                        if do_act and thr > 0:
                            tensor.wait_ge(asem, thr)
                        for t in (2 * h, 2 * h + 1):
                            for jh in range(2):
                                off = 1024 * t + 512 * jh
                                tensor.matmul(
                                    ps[:, off : off + 512],
                                    mv2[:, P * t : P * (t + 1)],
                                    s2[:, 512 * jh : 512 * (jh + 1)],
                                    start=True,
                                    stop=True,
                                ).then_inc(tsem)

        if do_act:

            @block.scalar
            def _(scalar):
                scalar.wait_ge(constsem, 2)
                for it in range(iters):
                    buf = it % NBT
                    for h in range(2):
                        scalar.wait_ge(tsem, 8 * it + 4 * (h + 1))
                        scalar.activation(
                            atn_sb[:, h * HW : (h + 1) * HW],
                            ps[:, h * HW : (h + 1) * HW],
                            AF.Arctan,
                            bias=B_,
                            scale=A_,
                        ).then_inc(asem)
                    if do_dve and it >= NBT:
                        # WAR: st[buf] was read by iteration it-NBT's DVE
                        scalar.wait_ge(vsem, NV * (it - NBT + 1))
                    for h in range(2):
                        scalar.activation(
                            st_sb[buf][:, h * HW : (h + 1) * HW],
                            atn_sb[:, h * HW : (h + 1) * HW],
                            AF.Sigmoid,
                            bias=E_,
                            scale=C_,
                        ).then_inc(ssem)

        if use_pool:

            @block.gpsimd
            def _(gp):
                # Pool multiplies mask*st for tiles 2,3 (idle engine offload)
                for it in range(iters):
                    buf = it % NBT
                    mbuf = it % NBM if mask_every else 0
                    gp.wait_ge(dmsem, 16 * (it + 1) if mask_every else 16)
                    gp.wait_ge(ssem, 2 * (it + 1))
                    if it > 0:
                        # WAR: DVE(it-1) ms-tree level1 (6th instr) read ms
                        gp.wait_ge(vsem, NV * (it - 1) + 6)
                    gp.tensor_tensor(
                        out=ms_sb[:, 2048:4096],
                        in0=m_sb[mbuf][:, 2048:4096],
                        in1=st_sb[buf][:, 2048:4096],
                        op=A.mult,
                    ).then_inc(psem)

        if do_dve:

            @block.vector
            def _(vector):
                for it in range(iters):
                    buf = it % NBT
                    mbuf = it % NBM if mask_every else 0
                    vector.wait_ge(dmsem, 16 * (it + 1) if mask_every else 16)
                    if do_act:
                        vector.wait_ge(ssem, 2 * (it + 1))
                    elif do_mm:
                        vector.wait_ge(tsem, 8 * (it + 1))
                    st = st_sb[buf]
                    st3 = st.rearrange("p (t j) -> p t j", t=TI)
                    m3 = m_sb[mbuf]
                    ms3 = ms_sb.rearrange("p (t j) -> p t j", t=TI)
                    a1 = scr_sb[:, 0:2048].rearrange("p (t j) -> p t j", t=TI)
                    a2 = scr_sb[:, 2048:3072].rearrange(
                        "p (t j) -> p t j", t=TI
                    )
                    a3 = scr_sb[:, 3072:3584].rearrange(
                        "p (t j) -> p t j", t=TI
                    )
                    if use_stt:
                        # Sm per tile in ONE fused instr: accum((st*1)*m);
                        # the ms tensor and its tree disappear entirely
                        for t in range(TI):
                            sl = slice(1024 * t, 1024 * (t + 1))
                            vector.scalar_tensor_tensor(
                                out=scr_sb[:, sl],
                                in0=st[:, sl],
                                scalar=1.0,
                                in1=m3[:, sl],
                                op0=A.mult,
                                op1=A.mult,
                                accum_out=acc_sb[:, TI + t : TI + t + 1],
                            ).then_inc(vsem)
                        # Sall via the halving tree (unmasked path)
                        a1s = a1
                        vector.tensor_tensor(
                            out=a1s[:, :, :],
                            in0=st3[:, :, 0:512],
                            in1=st3[:, :, 512:1024],
                            op=A.add,
                        ).then_inc(vsem)
                        vector.tensor_tensor(
                            out=a2[:, :, :],
                            in0=a1s[:, :, 0:256],
                            in1=a1s[:, :, 256:512],
                            op=A.add,
                        ).then_inc(vsem)
                        vector.tensor_tensor(
                            out=a3[:, :, :],
                            in0=a2[:, :, 0:128],
                            in1=a2[:, :, 128:256],
                            op=A.add,
                        ).then_inc(vsem)
                        vector.tensor_reduce(
                            acc_sb[:, 0:TI],
                            a3[:, :, :],
                            mybir.AxisListType.X,
                            A.add,
                        ).then_inc(vsem)
                        # keep vsem at NV increments per iteration
                        vector.memset(scr_sb[0:1, 0:1], 0).then_inc(vsem)
                        continue
                    # ms = mask * st (tiles 0-1 here; 2-3 on Pool if enabled)
                    if use_pool:
                        vector.tensor_tensor(
                            out=ms_sb[:, 0:2048],
                            in0=m3[:, 0:2048],
                            in1=st[:, 0:2048],
                            op=A.mult,
                        ).then_inc(vsem)
                    else:
                        vector.tensor_tensor(
                            out=ms_sb[:, :], in0=m3[:, :], in1=st[:, :],
                            op=A.mult,
                        ).then_inc(vsem)
                    # per-tile free-axis sums via halving tree + one
                    # innermost-axis tensor_reduce for all 4 tiles at once
                    for (src3, col) in ((st3, 0), (ms3, TI)):
                        if use_pool and src3 is ms3:
                            vector.wait_ge(psem, it + 1)
                        vector.tensor_tensor(
                            out=a1[:, :, :],
                            in0=src3[:, :, 0:512],
                            in1=src3[:, :, 512:1024],
                            op=A.add,
                        ).then_inc(vsem)
                        vector.tensor_tensor(
                            out=a2[:, :, :],
                            in0=a1[:, :, 0:256],
                            in1=a1[:, :, 256:512],
                            op=A.add,
                        ).then_inc(vsem)
                        vector.tensor_tensor(
                            out=a3[:, :, :],
                            in0=a2[:, :, 0:128],
                            in1=a2[:, :, 128:256],
                            op=A.add,
                        ).then_inc(vsem)
                        vector.tensor_reduce(
                            acc_sb[:, col : col + TI],
                            a3[:, :, :],
                            mybir.AxisListType.X,
                            A.add,
                        ).then_inc(vsem)


def _build_nc(iters=1, variant="full"):
    import concourse.bass as bass
    import concourse.mybir as mybir

    nc = bass.Bass()

    # Register const APs for the two activation biases (framework only
    # pre-registers 0.0/1.0). Only ScalarE reads these; a semaphore from the
    # memsets suffices (no all-engine barrier).
    constsem = nc.alloc_semaphore("constsem")
    for v in (B_, E_):
        t = nc.alloc_sbuf_tensor(f"const-f32-{v}", [128, 1], mybir.dt.float32)
        nc.gpsimd.memset(t.ap(), v).then_inc(constsem)
        nc.const_aps.aps[(mybir.dt.float32, v)] = t.ap()

    f32, bf16 = mybir.dt.float32, mybir.dt.bfloat16
    atn_dt = mybir.dt.float16 if variant == "f16" else f32
    ext = (
        nc.declare_dram_parameter("smv", [K2, SMV_W], bf16, isOutput=False),
        nc.declare_dram_parameter("mask", [P, TI * N], bf16, isOutput=False),
        nc.declare_dram_parameter("acc", [P, 2 * TI], f32, isOutput=True),
    )
    smv_sb = [
        nc.alloc_sbuf_tensor(f"smv{i}", [K2, SMV_W], bf16) for i in range(NBS)
    ]
    m_sb = [
        nc.alloc_sbuf_tensor(f"m{i}", [P, TI * N], bf16) for i in range(NBM)
    ]
    st_sb = [
        nc.alloc_sbuf_tensor(f"st{i}", [P, TI * N], bf16) for i in range(NBT)
    ]
    sb = (
        [t.ap() for t in smv_sb],
        [t.ap() for t in m_sb],
        nc.alloc_sbuf_tensor("atn", [P, TI * N], atn_dt).ap(),
        [t.ap() for t in st_sb],
        nc.alloc_sbuf_tensor("ms", [P, TI * N], bf16).ap(),
        nc.alloc_sbuf_tensor("scr", [P, TI * N], bf16).ap(),
        nc.alloc_sbuf_tensor("acc_sb", [P, 2 * TI], f32).ap(),
        nc.alloc_psum_tensor("ps", [P, TI * N], f32).ap(),
    )
    # fix list-of-AP indexing helpers
    smv_l, m_l, atn, st_l, ms, scr, acc, ps = sb

    class _Pair:
        def __init__(self, aps):
            self.aps = aps

        def __getitem__(self, i):
            return self.aps[i]

    sb = (_Pair(smv_l), _Pair(m_l), atn, _Pair(st_l), ms, scr, acc, ps)

    sems = tuple(
        nc.alloc_semaphore(n)
        for n in ("dssem", "dmsem", "tsem", "asem", "ssem", "vsem", "osem",
                  "psem")
    ) + (constsem,)
    (dssem, dmsem, tsem, asem, ssem, vsem, osem, psem, constsem) = sems

    _build_body(nc, mybir, ext, sb, sems, iters, variant=variant)

    return nc


def _get_nc(iters=1, variant="full"):
    key = (iters, variant)
    if key not in _nc_cache:
        _nc_cache[key] = _build_nc(iters, variant)
    return _nc_cache[key]


def _split_bf16(a):
    hi = a.astype(ml_dtypes.bfloat16)
    lo = (a - hi.astype(np.float64)).astype(ml_dtypes.bfloat16)
    return hi, lo


def _host_prep(embedding, abs_coords, patch_mask):
    """Build per-core input maps."""
    x = embedding.astype(np.float64).copy()  # [B,N,E]
    x[:, :, :2] += abs_coords.astype(np.float64)
    r = np.einsum("bne,bne->bn", x, x)  # [B,N]

    in_maps = []
    for c in range(N_CORES):
        b, i0 = c // 2, ROWS * (c % 2)
        xt = x[b].T  # [E, N]
        s = np.empty((K, N), np.float64)
        s[:E] = xt
        s[E] = r[b]
        s[E + 1] = 1.0
        mv = np.empty((K, ROWS), np.float64)
        mv[:E] = -2.0 * xt[:, i0 : i0 + ROWS]
        mv[E] = 1.0
        mv[E + 1] = r[b, i0 : i0 + ROWS] + EPS
        s_hi, s_lo = _split_bf16(s)
        mv_hi, mv_lo = _split_bf16(mv)
        s2 = np.concatenate([s_hi, s_hi, s_lo], axis=0)      # [K2, N]
        mv2 = np.concatenate([mv_hi, mv_lo, mv_hi], axis=0)  # [K2, ROWS]
        smv = np.concatenate([mv2, s2], axis=1)              # [K2, SMV_W]
        # m_sb[p, 1024*t + j] = mask[b, i0 + 128*t + p, j]
        m = (
            patch_mask[b][i0 : i0 + ROWS, :]
            .reshape(TI, P, N)
            .transpose(1, 0, 2)
            .reshape(P, TI * N)
            .astype(ml_dtypes.bfloat16)
        )
        in_maps.append(
            {"smv": np.ascontiguousarray(smv), "mask": np.ascontiguousarray(m)}
        )
    return in_maps


def _host_combine(results, patch_mask):
    """Per-row logs + final sum on host (4096 rows, trivial)."""
    # device diagonal value: ssq_ii = EPS exactly
    s_ii = 1.0 / (1.0 + np.exp(-(C_ * np.arctan(A_ * EPS + B_) + E_)))

    total = 0.0
    for c in range(N_CORES):
        b, i0 = c // 2, ROWS * (c % 2)
        acc = results[c]["acc"].astype(np.float64)  # [128, 8]
        mrows = patch_mask[b][i0 : i0 + ROWS, :].astype(np.float64)
        npos_all = mrows.sum(axis=1)                     # [512] in row order
        diag_all = np.diagonal(patch_mask[b])[i0 : i0 + ROWS].astype(
            np.float64
        )
        for t in range(TI):
            sall = acc[:, t]
            sm = acc[:, TI + t]
            rows = slice(128 * t, 128 * (t + 1))
            npos = npos_all[rows]
            diag = diag_all[rows]
            sneg = sall - sm - (1.0 - diag) * s_ii
            negsum = (N - npos - (1.0 - diag)) + AL1 * sneg
            L = np.log(negsum)
            total += (npos * L - sm + (npos + AL1 * sm) / negsum).sum()
    return total


def _make_runner(nc, in_maps):
    """Persistent jitted SPMD runner mirroring bass2jax.run_bass_via_pjrt.

    Returns f() -> list[dict[name, np.ndarray]]; repeated calls reuse the
    compiled executable so wall-clock deltas reflect device execution.
    """
    import jax
    from jax.sharding import Mesh, PartitionSpec, NamedSharding
    from jax.experimental.shard_map import shard_map
    import concourse.mybir as mybir
    from concourse import bass2jax

    bass2jax.install_neuronx_cc_hook()
    nc.finalize()

    partition_name = nc.partition_id_tensor.name if nc.partition_id_tensor else None
    in_names, out_names, out_avals, zero_outs = [], [], [], []
    for alloc in nc.m.functions[0].allocations:
        if not isinstance(alloc, mybir.MemoryLocationSet):
            continue
        name = alloc.memorylocations[0].name
        if alloc.kind == "ExternalInput":
            if name != partition_name:
                in_names.append(name)
        elif alloc.kind == "ExternalOutput":
            shape = tuple(alloc.tensor_shape)
            dtype = mybir.dt.np(alloc.dtype)
            out_names.append(name)
            out_avals.append(jax.core.ShapedArray(shape, dtype))
            zero_outs.append(np.zeros(shape, dtype))
    n_params = len(in_names)
    n_outs = len(out_avals)
    in_names_all = in_names + out_names
    if partition_name is not None:
        in_names_all.append(partition_name)

    def _body(*args):
        operands = list(args)
        if partition_name is not None:
            operands.append(bass2jax.partition_id_tensor())
        outs = bass2jax._bass_exec_p.bind(
            *operands,
            out_avals=tuple(out_avals),
            in_names=tuple(in_names_all),
            out_names=tuple(out_names),
            lowering_input_output_aliases=(),
            sim_require_finite=True,
            sim_require_nnan=True,
            nc=nc,
        )
        return tuple(outs)

    devices = jax.devices()[:N_CORES]
    mesh = Mesh(np.asarray(devices), ("core",))
    in_specs = (PartitionSpec("core"),) * (n_params + n_outs)
    out_specs = (PartitionSpec("core"),) * len(out_names)
    sharded = jax.jit(
        shard_map(
            _body, mesh=mesh, in_specs=in_specs, out_specs=out_specs, check_rep=False
        ),
        keep_unused=True,
    )
    per_core = [[np.asarray(m[name]) for name in in_names] for m in in_maps]
    concat_in = [
        np.concatenate([per_core[c][i] for c in range(N_CORES)], axis=0)
        for i in range(n_params)
    ]
    shard = NamedSharding(mesh, PartitionSpec("core"))
    concat_in_dev = [jax.device_put(a, shard) for a in concat_in]

    concat_zeros_dev = [
        jax.device_put(
            np.zeros((N_CORES * z.shape[0], *z.shape[1:]), z.dtype), shard
        )
        for z in zero_outs
    ]

    def run(fetch=True, block=True):
        out_arrs = sharded(*concat_in_dev, *concat_zeros_dev)
        if not fetch:
            if block:
                jax.block_until_ready(out_arrs)
                return None
            return out_arrs
        out_arrs = [np.asarray(a) for a in out_arrs]
        return [
            {
                name: out_arrs[i].reshape(N_CORES, *out_avals[i].shape)[c]
                for i, name in enumerate(out_names)
            }
            for c in range(N_CORES)
        ]

    return run


def _run(embedding, abs_coords, patch_mask, trace=False, variant="stt"):
    from concourse.bass_utils import run_bass_kernel_spmd

    nc = _get_nc(1, variant)
    in_maps = _host_prep(embedding, abs_coords, patch_mask)
    res = run_bass_kernel_spmd(
        nc, in_maps, core_ids=list(range(N_CORES)), trace=trace
    )
    total = _host_combine(res.results, patch_mask)
    return np.asarray(total, dtype=np.float32), res


def bench(embedding, abs_coords, patch_mask, iters=257, variant="stt"):
    """Measure per-iteration HW time: async-queue k executions of an
    iters-looped NEFF, block once; slope over k cancels dispatch noise."""
    import jax

    in_maps = _host_prep(embedding, abs_coords, patch_mask)
    key = (iters, variant)
    if key not in _runner_cache:
        _runner_cache[key] = _make_runner(_get_nc(iters, variant), in_maps)
    f = _runner_cache[key]
    out = f()  # warm-up + correctness output

    def batch(k):
        outs = None
        t0 = time.perf_counter()
        for _ in range(k):
            outs = f(fetch=False, block=False)
        jax.block_until_ready(outs)
        return time.perf_counter() - t0

    batch(2)
    t2 = min(batch(2) for _ in range(4))
    t10 = min(batch(10) for _ in range(4))
    ns = (t10 - t2) / (8 * iters) * 1e9
    return ns, out


def kernel(embedding, abs_coords, patch_mask):
    emb = np.asarray(embedding)
    coords = np.asarray(abs_coords)
    mask = np.asarray(patch_mask)
    # retry guard: first executions on this fleet occasionally glitch
    # transiently -- either a non-finite result or a device-unrecoverable
    # exception; both clear on retry
    last_err = None
    out = None
    for attempt in range(4):
        try:
            out, _ = _run(emb, coords, mask)
        except Exception as e:  # device-side transient; back off and retry
            last_err = e
            time.sleep(2.0 * (attempt + 1))
            continue
        if np.isfinite(out):
            return out
    if last_err is not None and out is None:
        raise last_err
    return out


# revision 14
# speedup vs baseline: 1.2502x; 1.0064x over previous
"""Trainium2 Bass kernel for nn_AnchorPlusLoss (8 NeuronCores, data-parallel).

Math (per batch b):
  x = embedding; x[..., :2] += abs_coords          # coords fold into first 2 dims
  ssq[i,j] = ||x_i - x_j||^2 + EPS  (Gram matmul, bf16 hi/lo split)
  sim = sigmoid(5 - sqrt(ssq))  approximated WITHOUT sqrt/exp:
      logit(ssq) = 5 - sqrt(ssq) ~ C*arctan(A*ssq + B) + E   (4-param fit,
      max abs sigma-domain err 2.5e-3 over the off-diagonal range d>=4)
  em1 = expm1(sim) ~ AL1*sim                                  (deg-1 fit; the
      masked/unmasked halves share elements so fit errors cancel in negsum)
  negsum_i = nneg_i + AL1*(sum_j sim - sum_m sim - diag corr)
  loss_i = npos_i*log(negsum_i) - sum_m sim + (npos_i + AL1*sum_m sim)/negsum_i
  (end-to-end rel err vs reference ~4e-4, tolerance 2e-2)

Device layout (per core = one (batch, 512-row half)): partitions = i
(4 tiles of 128 rows), free axis = j (1024). Engine split per iteration:
  PE:      8 matmuls  -> psum ssq [128, 4096] f32          (~3.4 us @mid)
  ScalarE: arctan x2 (2048-wide, from PSUM) -> atn f32
           sigmoid x2 -> st bf16; both funcs share ONE activation table set
           (sigmoid_and_others) => no per-iteration table loads  (~7.6 us)
  DVE:     Sm per tile via fused scalar_tensor_tensor+accum((st*1)*mask);
           Sall via a bf16 halving tree + one tensor_reduce(X)   (~7 us)
  out:     acc [128, 8] f32 (Sall/Sm per tile) DMA'd once at the end.
Sums over j are per-partition free-axis accumulator outputs -- the old
per-element Sqrt/Tanh/Exp passes and 24 reduction matmuls are gone.
"""

import sys
import time

sys.path.insert(0, "/opt/trn_rl_repo")

import numpy as np
import ml_dtypes

N_CORES = 8
B, N, E = 4, 1024, 32
ROWS = 512          # rows (i) per core
P = 128             # partitions
TI = ROWS // P      # 4 i-tiles per core
K = 34              # contraction: 32 emb + 2 augmentation rows
K2 = 3 * K          # hi/lo packed contraction
EPS = 0.25          # added to ssq so the diagonal never goes negative
SMV_W = ROWS + N    # packed operand width: mv2 (512) | s2 (1024)

# logit(u) = 5 - sqrt(u) ~ C_*arctan(A_*u + B_) + E_  (minimax fit, u >= 16)
A_ = 0.047240052
B_ = 4.49660097
C_ = -70.9121647
E_ = 99.0348340
AL1 = 1.3196475487382737   # expm1(s) ~ AL1*s over s in [0, 0.70]

NV = 9    # DVE instructions per iteration (vsem increments)
NBS = 4   # smv input buffers (prefetch depth)
NBM = 4   # mask input buffers
NBT = 4   # sigmoid-output buffers

_nc_cache = {}
_runner_cache = {}


def _build_body(nc, mybir, ext, sb, sems, iters, variant="full"):
    """Emit all iterations inside ONE Block: per-iteration Blocks would put
    an all-engine barrier between iterations, serializing the pipeline.
    Semaphores are shared across iterations with monotonic per-iteration
    threshold offsets. Cross-iteration WAR hazards are covered by
    double-buffered inputs (smv/mask/st) plus tsem/vsem waits on the
    producer engines; same-engine hazards rely on in-order execution.

    variant (for bottleneck experiments): "full"; "nodve" drops the DVE
    stage; "noact" drops the activations; "nomask"/"nosmv" load that input
    only once; "mm" = DMAs+matmuls only; "dma" = DMAs only."""
    AF = mybir.ActivationFunctionType
    A = mybir.AluOpType
    smv_ext, m_ext, acc_ext = ext
    (smv_sb, m_sb, atn_sb, st_sb, ms_sb, scr_sb, acc_sb, ps) = sb
    (dssem, dmsem, tsem, asem, ssem, vsem, osem, psem, constsem) = sems
    HW = 2048  # half width (2 i-tiles) for the big activations

    do_mm = variant in ("full", "nodve", "noact", "nomask", "nosmv", "mm",
                       "nodma", "f16", "pool", "stt", "stt4")
    do_act = variant in ("full", "nodve", "nomask", "nosmv", "nodma", "f16",
                        "pool", "stt", "stt4")
    do_dve = variant in ("full", "noact", "nomask", "nosmv", "nodma", "f16",
                        "pool", "stt", "stt4")
    mask_every = variant not in ("nomask", "nodma")
    smv_every = variant not in ("nosmv", "nodma")
    use_pool = variant == "pool"
    use_stt = variant in ("stt", "stt4")
    use_w4 = variant == "stt4"   # single 4096-wide activation per pass
    ssem_per = 1 if use_w4 else 2   # ssem increments per iteration

    def dve_ready(it):
        # threshold proving iteration `it`'s DVE block is complete
        return 12 * (it + 1)

    with nc.Block() as block:

        @block.sync
        def _(sync):
            # iteration 0's inputs, then per-iteration prefetch of it+1
            sync.dma_start(smv_sb[0][:, :], smv_ext[:, :]).then_inc(dssem, 16)
            sync.dma_start(m_sb[0][:, :], m_ext[:, :]).then_inc(dmsem, 16)
            for it in range(iters - 1):
                if smv_every:
                    # buffer (it+1)%NBS last read by PE(it+1-NBS)
                    thr = 8 * (it + 2 - NBS)
                    if thr > 0:
                        if do_mm:
                            sync.wait_ge(tsem, thr)
                        else:
                            sync.wait_ge(dssem, 2 * thr)
                    sync.dma_start(
                        smv_sb[(it + 1) % NBS][:, :], smv_ext[:, :]
                    ).then_inc(dssem, 16)
                if mask_every:
                    # buffer (it+1)%NBM last read by DVE(it+1-NBM) instr 1
                    k = it + 1 - NBM
                    if k >= 0:
                        if do_dve:
                            sync.wait_ge(vsem, NV * k + 1)
                        else:
                            sync.wait_ge(dmsem, 16 * (k + 1))
                    sync.dma_start(
                        m_sb[(it + 1) % NBM][:, :], m_ext[:, :]
                    ).then_inc(dmsem, 16)
            # epilogue: single output DMA once the last stage is done
            if do_dve:
                sync.wait_ge(vsem, NV * iters)
            elif do_act:
                sync.wait_ge(ssem, ssem_per * iters)
            elif do_mm:
                sync.wait_ge(tsem, 8 * iters)
            else:
                sync.wait_ge(dssem, 16 * (iters - 1) if smv_every else 16)
                sync.wait_ge(dmsem, 16 * (iters - 1) if mask_every else 16)
            sync.dma_start(acc_ext[:, :], acc_sb[:, :]).then_inc(osem, 16)
            sync.wait_ge(osem, 16)

        if do_mm:

            @block.tensor
            def _(tensor):
                for it in range(iters):
                    buf = it % NBS if smv_every else 0
                    tensor.wait_ge(
                        dssem, 16 * (it + 1) if smv_every else 16
                    )
                    mv2 = smv_sb[buf][:, 0:ROWS]
                    s2 = smv_sb[buf][:, ROWS : ROWS + N]
                    for h in range(2):
                        # WAR: ps half h read by iteration it-1's arctan h
                        if use_w4:
                            if h == 0 and it > 0:
                                tensor.wait_ge(asem, it)
                        else:
                            thr = 2 * (it - 1) + h + 1
                            if do_act and thr > 0:
                                tensor.wait_ge(asem, thr)
                        for t in (2 * h, 2 * h + 1):
                            for jh in range(2):
                                off = 1024 * t + 512 * jh
                                tensor.matmul(
                                    ps[:, off : off + 512],
                                    mv2[:, P * t : P * (t + 1)],
                                    s2[:, 512 * jh : 512 * (jh + 1)],
                                    start=True,
                                    stop=True,
                                ).then_inc(tsem)

        if do_act:

            @block.scalar
            def _(scalar):
                scalar.wait_ge(constsem, 2)
                for it in range(iters):
                    buf = it % NBT
                    if use_w4:
                        scalar.wait_ge(tsem, 8 * (it + 1))
                        scalar.activation(
                            atn_sb[:, :],
                            ps[:, :],
                            AF.Arctan,
                            bias=B_,
                            scale=A_,
                        ).then_inc(asem)
                        if do_dve and it >= NBT:
                            scalar.wait_ge(vsem, NV * (it - NBT + 1))
                        scalar.activation(
                            st_sb[buf][:, :],
                            atn_sb[:, :],
                            AF.Sigmoid,
                            bias=E_,
                            scale=C_,
                        ).then_inc(ssem)
                        continue
                    for h in range(2):
                        scalar.wait_ge(tsem, 8 * it + 4 * (h + 1))
                        scalar.activation(
                            atn_sb[:, h * HW : (h + 1) * HW],
                            ps[:, h * HW : (h + 1) * HW],
                            AF.Arctan,
                            bias=B_,
                            scale=A_,
                        ).then_inc(asem)
                    if do_dve and it >= NBT:
                        # WAR: st[buf] was read by iteration it-NBT's DVE
                        scalar.wait_ge(vsem, NV * (it - NBT + 1))
                    for h in range(2):
                        scalar.activation(
                            st_sb[buf][:, h * HW : (h + 1) * HW],
                            atn_sb[:, h * HW : (h + 1) * HW],
                            AF.Sigmoid,
                            bias=E_,
                            scale=C_,
                        ).then_inc(ssem)

        if use_pool:

            @block.gpsimd
            def _(gp):
                # Pool multiplies mask*st for tiles 2,3 (idle engine offload)
                for it in range(iters):
                    buf = it % NBT
                    mbuf = it % NBM if mask_every else 0
                    gp.wait_ge(dmsem, 16 * (it + 1) if mask_every else 16)
                    gp.wait_ge(ssem, 2 * (it + 1))
                    if it > 0:
                        # WAR: DVE(it-1) ms-tree level1 (6th instr) read ms
                        gp.wait_ge(vsem, NV * (it - 1) + 6)
                    gp.tensor_tensor(
                        out=ms_sb[:, 2048:4096],
                        in0=m_sb[mbuf][:, 2048:4096],
                        in1=st_sb[buf][:, 2048:4096],
                        op=A.mult,
                    ).then_inc(psem)

        if do_dve:

            @block.vector
            def _(vector):
                for it in range(iters):
                    buf = it % NBT
                    mbuf = it % NBM if mask_every else 0
                    vector.wait_ge(dmsem, 16 * (it + 1) if mask_every else 16)
                    if do_act:
                        vector.wait_ge(ssem, ssem_per * (it + 1))
                    elif do_mm:
                        vector.wait_ge(tsem, 8 * (it + 1))
                    st = st_sb[buf]
                    st3 = st.rearrange("p (t j) -> p t j", t=TI)
                    m3 = m_sb[mbuf]
                    ms3 = ms_sb.rearrange("p (t j) -> p t j", t=TI)
                    a1 = scr_sb[:, 0:2048].rearrange("p (t j) -> p t j", t=TI)
                    a2 = scr_sb[:, 2048:3072].rearrange(
                        "p (t j) -> p t j", t=TI
                    )
                    a3 = scr_sb[:, 3072:3584].rearrange(
                        "p (t j) -> p t j", t=TI
                    )
                    if use_stt:
                        # Sm per tile in ONE fused instr: accum((st*1)*m);
                        # the ms tensor and its tree disappear entirely
                        for t in range(TI):
                            sl = slice(1024 * t, 1024 * (t + 1))
                            vector.scalar_tensor_tensor(
                                out=scr_sb[:, sl],
                                in0=st[:, sl],
                                scalar=1.0,
                                in1=m3[:, sl],
                                op0=A.mult,
                                op1=A.mult,
                                accum_out=acc_sb[:, TI + t : TI + t + 1],
                            ).then_inc(vsem)
                        # Sall via the halving tree (unmasked path)
                        a1s = a1
                        vector.tensor_tensor(
                            out=a1s[:, :, :],
                            in0=st3[:, :, 0:512],
                            in1=st3[:, :, 512:1024],
                            op=A.add,
                        ).then_inc(vsem)
                        vector.tensor_tensor(
                            out=a2[:, :, :],
                            in0=a1s[:, :, 0:256],
                            in1=a1s[:, :, 256:512],
                            op=A.add,
                        ).then_inc(vsem)
                        vector.tensor_tensor(
                            out=a3[:, :, :],
                            in0=a2[:, :, 0:128],
                            in1=a2[:, :, 128:256],
                            op=A.add,
                        ).then_inc(vsem)
                        vector.tensor_reduce(
                            acc_sb[:, 0:TI],
                            a3[:, :, :],
                            mybir.AxisListType.X,
                            A.add,
                        ).then_inc(vsem)
                        # keep vsem at NV increments per iteration
                        vector.memset(scr_sb[0:1, 0:1], 0).then_inc(vsem)
                        continue
                    # ms = mask * st (tiles 0-1 here; 2-3 on Pool if enabled)
                    if use_pool:
                        vector.tensor_tensor(
                            out=ms_sb[:, 0:2048],
                            in0=m3[:, 0:2048],
                            in1=st[:, 0:2048],
                            op=A.mult,
                        ).then_inc(vsem)
                    else:
                        vector.tensor_tensor(
                            out=ms_sb[:, :], in0=m3[:, :], in1=st[:, :],
                            op=A.mult,
                        ).then_inc(vsem)
                    # per-tile free-axis sums via halving tree + one
                    # innermost-axis tensor_reduce for all 4 tiles at once
                    for (src3, col) in ((st3, 0), (ms3, TI)):
                        if use_pool and src3 is ms3:
                            vector.wait_ge(psem, it + 1)
                        vector.tensor_tensor(
                            out=a1[:, :, :],
                            in0=src3[:, :, 0:512],
                            in1=src3[:, :, 512:1024],
                            op=A.add,
                        ).then_inc(vsem)
                        vector.tensor_tensor(
                            out=a2[:, :, :],
                            in0=a1[:, :, 0:256],
                            in1=a1[:, :, 256:512],
                            op=A.add,
                        ).then_inc(vsem)
                        vector.tensor_tensor(
                            out=a3[:, :, :],
                            in0=a2[:, :, 0:128],
                            in1=a2[:, :, 128:256],
                            op=A.add,
                        ).then_inc(vsem)
                        vector.tensor_reduce(
                            acc_sb[:, col : col + TI],
                            a3[:, :, :],
                            mybir.AxisListType.X,
                            A.add,
                        ).then_inc(vsem)


def _build_nc(iters=1, variant="full"):
    import concourse.bass as bass
    import concourse.mybir as mybir

    nc = bass.Bass()

    # Register const APs for the two activation biases (framework only
    # pre-registers 0.0/1.0). Only ScalarE reads these; a semaphore from the
    # memsets suffices (no all-engine barrier).
    constsem = nc.alloc_semaphore("constsem")
    for v in (B_, E_):
        t = nc.alloc_sbuf_tensor(f"const-f32-{v}", [128, 1], mybir.dt.float32)
        nc.gpsimd.memset(t.ap(), v).then_inc(constsem)
        nc.const_aps.aps[(mybir.dt.float32, v)] = t.ap()

    f32, bf16 = mybir.dt.float32, mybir.dt.bfloat16
    atn_dt = mybir.dt.float16 if variant == "f16" else f32
    ext = (
        nc.declare_dram_parameter("smv", [K2, SMV_W], bf16, isOutput=False),
        nc.declare_dram_parameter("mask", [P, TI * N], bf16, isOutput=False),
        nc.declare_dram_parameter("acc", [P, 2 * TI], f32, isOutput=True),
    )
    smv_sb = [
        nc.alloc_sbuf_tensor(f"smv{i}", [K2, SMV_W], bf16) for i in range(NBS)
    ]
    m_sb = [
        nc.alloc_sbuf_tensor(f"m{i}", [P, TI * N], bf16) for i in range(NBM)
    ]
    st_sb = [
        nc.alloc_sbuf_tensor(f"st{i}", [P, TI * N], bf16) for i in range(NBT)
    ]
    sb = (
        [t.ap() for t in smv_sb],
        [t.ap() for t in m_sb],
        nc.alloc_sbuf_tensor("atn", [P, TI * N], atn_dt).ap(),
        [t.ap() for t in st_sb],
        nc.alloc_sbuf_tensor("ms", [P, TI * N], bf16).ap(),
        nc.alloc_sbuf_tensor("scr", [P, TI * N], bf16).ap(),
        nc.alloc_sbuf_tensor("acc_sb", [P, 2 * TI], f32).ap(),
        nc.alloc_psum_tensor("ps", [P, TI * N], f32).ap(),
    )
    # fix list-of-AP indexing helpers
    smv_l, m_l, atn, st_l, ms, scr, acc, ps = sb

    class _Pair:
        def __init__(self, aps):
            self.aps = aps

        def __getitem__(self, i):
            return self.aps[i]

    sb = (_Pair(smv_l), _Pair(m_l), atn, _Pair(st_l), ms, scr, acc, ps)

    sems = tuple(
        nc.alloc_semaphore(n)
        for n in ("dssem", "dmsem", "tsem", "asem", "ssem", "vsem", "osem",
                  "psem")
    ) + (constsem,)
    (dssem, dmsem, tsem, asem, ssem, vsem, osem, psem, constsem) = sems

    _build_body(nc, mybir, ext, sb, sems, iters, variant=variant)

    return nc


def _get_nc(iters=1, variant="full"):
    key = (iters, variant)
    if key not in _nc_cache:
        _nc_cache[key] = _build_nc(iters, variant)
    return _nc_cache[key]


def _split_bf16(a):
    hi = a.astype(ml_dtypes.bfloat16)
    lo = (a - hi.astype(np.float64)).astype(ml_dtypes.bfloat16)
    return hi, lo


def _host_prep(embedding, abs_coords, patch_mask):
    """Build per-core input maps."""
    x = embedding.astype(np.float64).copy()  # [B,N,E]
    x[:, :, :2] += abs_coords.astype(np.float64)
    r = np.einsum("bne,bne->bn", x, x)  # [B,N]

    in_maps = []
    for c in range(N_CORES):
        b, i0 = c // 2, ROWS * (c % 2)
        xt = x[b].T  # [E, N]
        s = np.empty((K, N), np.float64)
        s[:E] = xt
        s[E] = r[b]
        s[E + 1] = 1.0
        mv = np.empty((K, ROWS), np.float64)
        mv[:E] = -2.0 * xt[:, i0 : i0 + ROWS]
        mv[E] = 1.0
        mv[E + 1] = r[b, i0 : i0 + ROWS] + EPS
        s_hi, s_lo = _split_bf16(s)
        mv_hi, mv_lo = _split_bf16(mv)
        s2 = np.concatenate([s_hi, s_hi, s_lo], axis=0)      # [K2, N]
        mv2 = np.concatenate([mv_hi, mv_lo, mv_hi], axis=0)  # [K2, ROWS]
        smv = np.concatenate([mv2, s2], axis=1)              # [K2, SMV_W]
        # m_sb[p, 1024*t + j] = mask[b, i0 + 128*t + p, j]
        m = (
            patch_mask[b][i0 : i0 + ROWS, :]
            .reshape(TI, P, N)
            .transpose(1, 0, 2)
            .reshape(P, TI * N)
            .astype(ml_dtypes.bfloat16)
        )
        in_maps.append(
            {"smv": np.ascontiguousarray(smv), "mask": np.ascontiguousarray(m)}
        )
    return in_maps


def _host_combine(results, patch_mask):
    """Per-row logs + final sum on host (4096 rows, trivial)."""
    # device diagonal value: ssq_ii = EPS exactly
    s_ii = 1.0 / (1.0 + np.exp(-(C_ * np.arctan(A_ * EPS + B_) + E_)))

    total = 0.0
    for c in range(N_CORES):
        b, i0 = c // 2, ROWS * (c % 2)
        acc = results[c]["acc"].astype(np.float64)  # [128, 8]
        mrows = patch_mask[b][i0 : i0 + ROWS, :].astype(np.float64)
        npos_all = mrows.sum(axis=1)                     # [512] in row order
        diag_all = np.diagonal(patch_mask[b])[i0 : i0 + ROWS].astype(
            np.float64
        )
        for t in range(TI):
            sall = acc[:, t]
            sm = acc[:, TI + t]
            rows = slice(128 * t, 128 * (t + 1))
            npos = npos_all[rows]
            diag = diag_all[rows]
            sneg = sall - sm - (1.0 - diag) * s_ii
            negsum = (N - npos - (1.0 - diag)) + AL1 * sneg
            L = np.log(negsum)
            total += (npos * L - sm + (npos + AL1 * sm) / negsum).sum()
    return total


def _make_runner(nc, in_maps):
    """Persistent jitted SPMD runner mirroring bass2jax.run_bass_via_pjrt.

    Returns f() -> list[dict[name, np.ndarray]]; repeated calls reuse the
    compiled executable so wall-clock deltas reflect device execution.
    """
    import jax
    from jax.sharding import Mesh, PartitionSpec, NamedSharding
    from jax.experimental.shard_map import shard_map
    import concourse.mybir as mybir
    from concourse import bass2jax

    bass2jax.install_neuronx_cc_hook()
    nc.finalize()

    partition_name = nc.partition_id_tensor.name if nc.partition_id_tensor else None
    in_names, out_names, out_avals, zero_outs = [], [], [], []
    for alloc in nc.m.functions[0].allocations:
        if not isinstance(alloc, mybir.MemoryLocationSet):
            continue
        name = alloc.memorylocations[0].name
        if alloc.kind == "ExternalInput":
            if name != partition_name:
                in_names.append(name)
        elif alloc.kind == "ExternalOutput":
            shape = tuple(alloc.tensor_shape)
            dtype = mybir.dt.np(alloc.dtype)
            out_names.append(name)
            out_avals.append(jax.core.ShapedArray(shape, dtype))
            zero_outs.append(np.zeros(shape, dtype))
    n_params = len(in_names)
    n_outs = len(out_avals)
    in_names_all = in_names + out_names
    if partition_name is not None:
        in_names_all.append(partition_name)

    def _body(*args):
        operands = list(args)
        if partition_name is not None:
            operands.append(bass2jax.partition_id_tensor())
        outs = bass2jax._bass_exec_p.bind(
            *operands,
            out_avals=tuple(out_avals),
            in_names=tuple(in_names_all),
            out_names=tuple(out_names),
            lowering_input_output_aliases=(),
            sim_require_finite=True,
            sim_require_nnan=True,
            nc=nc,
        )
        return tuple(outs)

    devices = jax.devices()[:N_CORES]
    mesh = Mesh(np.asarray(devices), ("core",))
    in_specs = (PartitionSpec("core"),) * (n_params + n_outs)
    out_specs = (PartitionSpec("core"),) * len(out_names)
    sharded = jax.jit(
        shard_map(
            _body, mesh=mesh, in_specs=in_specs, out_specs=out_specs, check_rep=False
        ),
        keep_unused=True,
    )
    per_core = [[np.asarray(m[name]) for name in in_names] for m in in_maps]
    concat_in = [
        np.concatenate([per_core[c][i] for c in range(N_CORES)], axis=0)
        for i in range(n_params)
    ]
    shard = NamedSharding(mesh, PartitionSpec("core"))
    concat_in_dev = [jax.device_put(a, shard) for a in concat_in]

    concat_zeros_dev = [
        jax.device_put(
            np.zeros((N_CORES * z.shape[0], *z.shape[1:]), z.dtype), shard
        )
        for z in zero_outs
    ]

    def run(fetch=True, block=True):
        out_arrs = sharded(*concat_in_dev, *concat_zeros_dev)
        if not fetch:
            if block:
                jax.block_until_ready(out_arrs)
                return None
            return out_arrs
        out_arrs = [np.asarray(a) for a in out_arrs]
        return [
            {
                name: out_arrs[i].reshape(N_CORES, *out_avals[i].shape)[c]
                for i, name in enumerate(out_names)
            }
            for c in range(N_CORES)
        ]

    return run


def _run(embedding, abs_coords, patch_mask, trace=False, variant="stt"):
    from concourse.bass_utils import run_bass_kernel_spmd

    nc = _get_nc(1, variant)
    in_maps = _host_prep(embedding, abs_coords, patch_mask)
    res = run_bass_kernel_spmd(
        nc, in_maps, core_ids=list(range(N_CORES)), trace=trace
    )
    total = _host_combine(res.results, patch_mask)
    return np.asarray(total, dtype=np.float32), res


def bench(embedding, abs_coords, patch_mask, iters=257, variant="stt"):
    """Measure per-iteration HW time: async-queue k executions of an
    iters-looped NEFF, block once; slope over k cancels dispatch noise."""
    import jax

    in_maps = _host_prep(embedding, abs_coords, patch_mask)
    key = (iters, variant)
    if key not in _runner_cache:
        _runner_cache[key] = _make_runner(_get_nc(iters, variant), in_maps)
    f = _runner_cache[key]
    out = f()  # warm-up + correctness output

    def batch(k):
        outs = None
        t0 = time.perf_counter()
        for _ in range(k):
            outs = f(fetch=False, block=False)
        jax.block_until_ready(outs)
        return time.perf_counter() - t0

    batch(2)
    t2 = min(batch(2) for _ in range(4))
    t10 = min(batch(10) for _ in range(4))
    ns = (t10 - t2) / (8 * iters) * 1e9
    return ns, out


def kernel(embedding, abs_coords, patch_mask):
    emb = np.asarray(embedding)
    coords = np.asarray(abs_coords)
    mask = np.asarray(patch_mask)
    # retry guard: first executions on this fleet occasionally glitch
    # transiently -- either a non-finite result or a device-unrecoverable
    # exception; both clear on retry
    last_err = None
    out = None
    for attempt in range(4):
        try:
            out, _ = _run(emb, coords, mask)
        except Exception as e:  # device-side transient; back off and retry
            last_err = e
            time.sleep(2.0 * (attempt + 1))
            continue
        if np.isfinite(out):
            return out
    if last_err is not None and out is None:
        raise last_err
    return out
